# revision 33
# baseline (speedup 1.0000x reference)
"""Trainium2 Bass kernel for nn_Controller (ENAS-style NAS controller).

The reference is a strictly sequential batch-1 LSTM controller (D=64, 54 cell
steps across two sampler calls) with data-dependent categorical sampling
(jax.random.categorical with keys derived from jax.random.key(42)) feeding
back into the LSTM input.  The RNG keys are compile-time constants of the
model, so categorical(key, logits) == argmax(logits + gumbel(key)) with a
precomputed Gumbel table (embedded below as exact fp32 bit patterns).

Device mapping highlights:
  - everything lives in column layout on partitions 0:63 (state h, c, x)
  - sigmoid(z) = 0.5*(1 + tanh(z/2)); the 0.5 pre-scale is folded into the
    weights so ONE tanh activation computes all four gates [f;i | o;g] in a
    [128,2] PSUM tile; the (t+1)/2 post-form is folded into the DVE ops and
    the h/c state is kept doubled (h_dbl = 2h, s = 2c)
  - the cross-partition reduction sig_f*c + sig_i*tanh(g) is done with one
    tiny matmul against a constant [0.5*I; I] selector
  - per-sample softmax stats avoid Ln: a running product of (S * 0.25) is
    kept and a single Ln at the end produces sum(log S)
  - argmax via DVE max on an 8-wide padded row; selection is a one-hot row
    (glg == max), transposed to a column with a K=1 matmul and applied as a
    gather matmul against row-stored anchors / embedding tables

Not shardable (batch=1, sequential): the program is replicated SPMD on all 8
cores; core 0's output is returned.
"""

import sys
import numpy as np

for _p in ("/opt/trn_rl_repo", "/root/.axon_site/_ro/trn_rl_repo"):
    if _p not in sys.path:
        sys.path.append(_p)

D = 64
KSCALE = 0.25           # per-step scale on the running product of softmax sums
NSTEPS = 40

GUMBEL_BITS = [
    0xbf9a9a59, 0xbe04727b, 0xf149f2ca, 0xf149f2ca, 0xf149f2ca, 0xf149f2ca, 0xf149f2ca, 0xf149f2ca,
    0xbe6342c0, 0x3fd214da, 0xf149f2ca, 0xf149f2ca, 0xf149f2ca, 0xf149f2ca, 0xf149f2ca, 0xf149f2ca,
    0xbfa75915, 0xbe3f2b11, 0x3fe94f7f, 0x4030acc3, 0x3f4e75bb, 0xf149f2ca, 0xf149f2ca, 0xf149f2ca,
    0xbdf262df, 0x400f470b, 0x3f02fdf4, 0xbd0bd8d2, 0x40527a81, 0xf149f2ca, 0xf149f2ca, 0xf149f2ca,
    0x4002b618, 0x3ec58d7c, 0xbe91bcb8, 0xf149f2ca, 0xf149f2ca, 0xf149f2ca, 0xf149f2ca, 0xf149f2ca,
    0x3eda16dc, 0xbe5832f5, 0x3fd686e6, 0xf149f2ca, 0xf149f2ca, 0xf149f2ca, 0xf149f2ca, 0xf149f2ca,
    0xbf020c87, 0xbf36ff4f, 0xbfbf2d42, 0xbe843dd6, 0x3fc66336, 0xf149f2ca, 0xf149f2ca, 0xf149f2ca,
    0x3f22cf96, 0x3e27c77e, 0x3f330636, 0x3e76c467, 0x3f6501bd, 0xf149f2ca, 0xf149f2ca, 0xf149f2ca,
    0x4042470c, 0x3f5d4699, 0xbf846756, 0xbc0271b6, 0xf149f2ca, 0xf149f2ca, 0xf149f2ca, 0xf149f2ca,
    0x3da7b1d7, 0x3f008aa9, 0xbd982586, 0x4025eab9, 0xf149f2ca, 0xf149f2ca, 0xf149f2ca, 0xf149f2ca,
    0x3fbfb5e6, 0x3fe7d80c, 0x403a1d1c, 0x3f5a1419, 0xbf8a2fb7, 0xf149f2ca, 0xf149f2ca, 0xf149f2ca,
    0x3f65a079, 0xbf6ce82c, 0x3dfbbc0b, 0x3f0f8470, 0x3d265891, 0xf149f2ca, 0xf149f2ca, 0xf149f2ca,
    0x3f206107, 0x3e170fc6, 0x3f28cf54, 0x3e5f9846, 0x3f578214, 0xf149f2ca, 0xf149f2ca, 0xf149f2ca,
    0x3f88cba9, 0xbf139cb3, 0xbf61332a, 0x3e2ee286, 0x3f3e2556, 0xf149f2ca, 0xf149f2ca, 0xf149f2ca,
    0x40ae936d, 0x3f9a7c61, 0xbec4397d, 0xbee3d787, 0xbf1bb597, 0xf149f2ca, 0xf149f2ca, 0xf149f2ca,
    0xbf3f4d87, 0x3eb0223e, 0x3f94f781, 0xbee58edd, 0xbf1884c3, 0xf149f2ca, 0xf149f2ca, 0xf149f2ca,
    0x400398d8, 0x3eca3814, 0xbe86dad7, 0x3fb47f21, 0x3fd04687, 0x400f3b1a, 0xf149f2ca, 0xf149f2ca,
    0xbf637fde, 0x3e21b3bd, 0x3f2b06b1, 0x3e530fd1, 0x3f46edba, 0xbfca341b, 0xf149f2ca, 0xf149f2ca,
    0xbecd80e9, 0xbef53e19, 0xbf27de91, 0xbf998050, 0xbe076736, 0xf149f2ca, 0xf149f2ca, 0xf149f2ca,
    0x3ffef78a, 0x40a907ac, 0x3f9a730c, 0xbed47529, 0xbeefcc04, 0xf149f2ca, 0xf149f2ca, 0xf149f2ca,
    0x3dd18a54, 0x3f0c3f6f, 0xf149f2ca, 0xf149f2ca, 0xf149f2ca, 0xf149f2ca, 0xf149f2ca, 0xf149f2ca,
    0xbe4e05ba, 0x3fde815d, 0xf149f2ca, 0xf149f2ca, 0xf149f2ca, 0xf149f2ca, 0xf149f2ca, 0xf149f2ca,
    0x3ee320e2, 0xbe3636cb, 0x3ff04e67, 0x40455de2, 0x3f65efbc, 0xf149f2ca, 0xf149f2ca, 0xf149f2ca,
    0x3fb8f968, 0x3fd712e0, 0x40156552, 0x3f182e1e, 0x3da14508, 0xf149f2ca, 0xf149f2ca, 0xf149f2ca,
    0x4037a6a2, 0x3f4e4715, 0xbfa30a18, 0xf149f2ca, 0xf149f2ca, 0xf149f2ca, 0xf149f2ca, 0xf149f2ca,
    0xbee2e632, 0xbf115ff5, 0xbf60ad54, 0xf149f2ca, 0xf149f2ca, 0xf149f2ca, 0xf149f2ca, 0xf149f2ca,
    0x3f479314, 0xbfb90a66, 0xbe73aba1, 0x3fbe48a5, 0x3fee0218, 0xf149f2ca, 0xf149f2ca, 0xf149f2ca,
    0xbf74e6c7, 0x3da87a1f, 0x3eff7832, 0xbdc44f5f, 0x4017966b, 0xf149f2ca, 0xf149f2ca, 0xf149f2ca,
    0x3cc8c6e0, 0x3ec14330, 0xbe9f74ef, 0x3fa2b3f3, 0xf149f2ca, 0xf149f2ca, 0xf149f2ca, 0xf149f2ca,
    0xbec23797, 0xbedddef0, 0xbf0adc47, 0xbf5a5a1a, 0xf149f2ca, 0xf149f2ca, 0xf149f2ca, 0xf149f2ca,
    0x3fc1c3c5, 0x3fed3255, 0x4048b461, 0x3f646639, 0xbf80e730, 0xf149f2ca, 0xf149f2ca, 0xf149f2ca,
    0xbf202b36, 0xbf859edb, 0xbc1874d9, 0x4068679e, 0x3f8137e2, 0xf149f2ca, 0xf149f2ca, 0xf149f2ca,
    0x3d611b7d, 0x3ee2cdd2, 0xbe330943, 0x3fe3f244, 0x4041794b, 0xf149f2ca, 0xf149f2ca, 0xf149f2ca,
    0x40854d84, 0x3f8e0f29, 0xbf021487, 0xbf304685, 0xbfa5d60a, 0xf149f2ca, 0xf149f2ca, 0xf149f2ca,
    0x3f265691, 0x3e3ff7f2, 0x3f40d297, 0x3ea7369e, 0x3f958039, 0xf149f2ca, 0xf149f2ca, 0xf149f2ca,
    0xbf6630be, 0x3e15278c, 0x3f23dbce, 0x3e2ba5ad, 0x3f2875cb, 0xf149f2ca, 0xf149f2ca, 0xf149f2ca,
    0x3e0651a1, 0x3f1d22fb, 0x3df34332, 0x3f121026, 0x3d1accf1, 0x3ec60c8b, 0xf149f2ca, 0xf149f2ca,
    0x3d99ebfe, 0x3ef97631, 0xbdd69458, 0x401ee5ed, 0x3f1f6d8e, 0x3dff0042, 0xf149f2ca, 0xf149f2ca,
    0xbf1c399f, 0xbf7e11d7, 0x3d49e37c, 0x3ee16a07, 0xbe1ec54e, 0xf149f2ca, 0xf149f2ca, 0xf149f2ca,
    0x3e70566f, 0x3f5dffd8, 0xbf7f9609, 0x3d302fcb, 0x3ee5e50e, 0xf149f2ca, 0xf149f2ca, 0xf149f2ca,
]

def gumbel_table():
    return np.array(GUMBEL_BITS, dtype=np.uint32).view(np.float32).reshape(40, 8)


# ---------------------------------------------------------------------------
# Host-side input preprocessing (weight folding).
# ---------------------------------------------------------------------------
def prep_inputs(inputs):
    f32 = np.float32
    p = {k: np.ascontiguousarray(np.asarray(v), dtype=f32) for k, v in inputs.items()}
    w_ih, w_hh = p['w_ih'], p['w_hh']
    b = (p['b_ih'] + p['b_hh']).astype(f32)
    gi, gf, gg_, go = (slice(0, 64), slice(64, 128), slice(128, 192), slice(192, 256))

    def stack_gates(W, scale_h):
        c0 = np.concatenate([0.5 * W[gf], 0.5 * W[gi]], axis=0)
        c1 = np.concatenate([0.5 * W[go], 1.0 * W[gg_]], axis=0)
        return (scale_h * c0).T.copy(), (scale_h * c1).T.copy()

    lx0, lx1 = stack_gates(w_ih, 1.0)
    lh0, lh1 = stack_gates(w_hh, 0.5)
    bias0 = np.concatenate([0.5 * b[gf], 0.5 * b[gi]])
    bias1 = np.concatenate([0.5 * b[go], 1.0 * b[gg_]])
    lx0 = np.concatenate([lx0, bias0[None]], axis=0).astype(f32)   # [65,128]
    lx1 = np.concatenate([lx1, bias1[None]], axis=0).astype(f32)   # [65,128]

    comb = np.concatenate([0.5 * np.eye(D, dtype=f32),
                           1.0 * np.eye(D, dtype=f32)], axis=0)    # [128,64]
    w1T = (0.5 * p['w_attn_1']).T.copy().astype(f32)               # [64,64]
    w2T = (0.5 * p['w_attn_2']).T.copy().astype(f32)               # [64,64]
    v5 = (0.2 * p['v_attn'].T).copy().astype(f32)                  # [64,1]
    wsoft5 = np.concatenate([(0.1 * p['w_soft']).T,
                             (0.2 * p['b_soft'])], axis=0).astype(f32)  # [65,5]
    emb = p['embed_w'].T.copy().astype(f32)                        # [64,6]
    embT1 = p['embed_w'][1:6].copy().astype(f32)                   # [5,64] rows 1..5
    G = gumbel_table().reshape(1, 320)
    brow = np.zeros((1, 16), f32)
    brow[0, :5] = p['b_soft_no_learn'][0]
    consts = np.zeros((1, 8), f32)
    consts[0, 0] = KSCALE
    consts[0, 1] = 1.0
    iden = np.eye(D, dtype=f32)
    urow = np.zeros((1, 64), f32)
    for s_ in range(8):
        urow[0, 8 * s_ + s_] = 1.0
    return dict(lx0=lx0, lx1=lx1, lh0=lh0.astype(f32), lh1=lh1.astype(f32),
                comb=comb, w1T=w1T, w2T=w2T, v5=v5, wsoft5=wsoft5, emb=emb,
                embT1=embT1, G=G, brow=brow, consts=consts, iden=iden, urow=urow)


# ---------------------------------------------------------------------------
# Program builder: two-phase emission with a minimal dependency tracker.
# ---------------------------------------------------------------------------
def build_program(strict=False):
    # strict: True -> same-engine waits everywhere (race-detector clean);
    #         a set/frozenset of engine names -> same-engine waits only there;
    #         False -> cross-engine waits only.
    if strict is True:
        strict_engs = {"pe", "act", "dve", "sp"}
    elif strict:
        strict_engs = set(strict)
    else:
        strict_engs = set()
    import concourse.bass as bass
    import concourse.mybir as mybir
    from contextlib import ExitStack

    AF = mybir.ActivationFunctionType
    ALU = mybir.AluOpType
    f32 = mybir.dt.float32

    nc = bass.Bass()

    # ---- external I/O ----
    ins = {}
    for name, shape in [("lx0", [65, 128]), ("lx1", [65, 128]),
                        ("lh0", [64, 128]), ("lh1", [64, 128]),
                        ("comb", [128, 64]), ("w1T", [64, 64]), ("w2T", [64, 64]),
                        ("v5", [64, 1]), ("wsoft5", [65, 5]), ("emb", [64, 6]),
                        ("embT1", [5, 64]), ("iden", [64, 64]), ("urow", [1, 64]),
                        ("G", [1, 320]), ("brow", [1, 16]), ("consts", [1, 8])]:
        ins[name] = nc.declare_dram_parameter(name, shape, f32, isOutput=False)
    arc_out = nc.declare_dram_parameter("arc", [1, 40], mybir.dt.int32, isOutput=True)
    scal_out = nc.declare_dram_parameter("scal", [1, 2], f32, isOutput=True)

    stack = ExitStack()
    sb = {}
    for name, shape, dt in [
        ("lx0", [65, 128], f32), ("lx1", [65, 128], f32),
        ("lh0", [64, 128], f32), ("lh1", [64, 128], f32),
        ("comb", [128, 64], f32), ("w1T", [64, 64], f32), ("w2T", [64, 64], f32),
        ("v5", [64, 1], f32), ("wsoft5", [65, 5], f32), ("emb", [64, 6], f32),
        ("embT1", [5, 64], f32), ("iden", [64, 64], f32), ("urow", [1, 64], f32),
        ("hrow", [1, 64], f32),
        ("G", [1, 320], f32), ("brow", [1, 16], f32), ("consts", [1, 8], f32),
        ("x", [65, 1], f32), ("h", [65, 1], f32),
        ("tS", [128, 2], f32), ("a2b1", [128, 1], f32), ("th", [64, 1], f32),
        ("hw2", [64, 1], f32), ("q", [64, 8], f32),
        ("aw1", [64, 8], f32), ("anchrows", [8, 64], f32),
        ("t1row", [1, 8], f32), ("lg8", [1, 8], f32), ("glg8", [1, 8], f32),
        ("mx8", [1, 8], f32), ("ix8", [1, 8], mybir.dt.uint32),
        ("oh8", [1, 8], f32), ("sboh", [8, 1], f32),
        ("e8", [1, 8], f32), ("S", [1, 1], f32), ("rS", [1, 1], f32),
        ("dotn", [1, 1], f32), ("dscr", [1, 8], f32), ("sel", [1, 1], f32),
        ("PS", [1, 1], f32), ("LG", [1, 1], f32), ("ENT", [1, 1], f32),
        ("lnPS", [1, 1], f32),
        ("arcrow", [1, 40], mybir.dt.uint32), ("scal", [1, 2], f32),
    ]:
        sb[name] = stack.enter_context(nc.sbuf_tensor("sb_" + name, shape, dt))

    ps = {}
    for name, shape in [("ps_g", [128, 2]), ("ps_s", [64, 1]), ("ps_hw2", [64, 1]),
                        ("ps_row", [1, 8]),
                        ("ps_oh", [8, 1]), ("ps_x", [64, 1]), ("ps_t", [1, 64]),
                        ("ps_rows", [8, 64])]:
        ps[name] = stack.enter_context(nc.psum_tensor(name, shape, f32))

    # ---- semaphores ----
    ENGS = ("pe", "act", "dve", "sp")
    sems = {e: stack.enter_context(nc.semaphore(f"sem_{e}")) for e in ENGS}
    sems["dma"] = stack.enter_context(nc.semaphore("sem_dma"))

    # ---- phase 1: abstract op list ----
    ops = []            # dicts: eng, fn(engine), reads, writes, self_wait, no_inc
    def op(eng, fn, reads=(), writes=(), self_wait=False, no_inc=False):
        ops.append(dict(eng=eng, fn=fn, reads=tuple(reads), writes=tuple(writes),
                        self_wait=self_wait, no_inc=no_inc, waits=[], idx=None))

    regs = {}           # filled at emit time ('idx' -> ScalarValue)

    # --- init: DMA loads ---
    for name in ins:
        def mk(name=name):
            def f(eng):
                return eng.dma_start(out=sb[name][:, :], in_=ins[name][:, :])
            return f
        op("sp", mk(), reads=(), writes=(name,))

    # --- init: memsets / constants ---
    def mset(buf, apfn, val):
        def f(eng):
            return eng.memset(apfn(), val)
        return f
    op("dve", mset("x", lambda: sb["x"][:, :], 0.0), writes=("x",))
    op("dve", mset("x64", lambda: sb["x"][64:65, :], 1.0), writes=("x",))
    op("dve", mset("h", lambda: sb["h"][:, :], 0.0), writes=("h",))
    op("dve", mset("h64", lambda: sb["h"][64:65, :], 1.0), writes=("h",))
    op("dve", mset("lg8", lambda: sb["lg8"][:, :], 0.0), writes=("lg8",))
    op("dve", mset("PS", lambda: sb["PS"][:, :], 1.0), writes=("PS",))
    op("dve", mset("LG", lambda: sb["LG"][:, :], 0.0), writes=("LG",))
    op("dve", mset("ENT", lambda: sb["ENT"][:, :], 0.0), writes=("ENT",))
    op("dve", mset("ps_s", lambda: ps["ps_s"][:, :], 0.0), writes=("ps_s",))
    op("dve", mset("anchrows", lambda: sb["anchrows"][:, :], 0.0), writes=("anchrows",))

    # --- building blocks ---
    def cell():
        op("pe", lambda e: e.matmul(ps["ps_g"][:, 0:1], sb["lx0"][:, :], sb["x"][:, :], start=True, stop=False),
           reads=("lx0", "x"), writes=("ps_g",))
        op("pe", lambda e: e.matmul(ps["ps_g"][:, 0:1], sb["lh0"][:, :], sb["h"][0:64, :], start=False, stop=True),
           reads=("lh0", "h"), writes=("ps_g",))
        op("pe", lambda e: e.matmul(ps["ps_g"][:, 1:2], sb["lx1"][:, :], sb["x"][:, :], start=True, stop=False),
           reads=("lx1", "x"), writes=("ps_g",))
        op("pe", lambda e: e.matmul(ps["ps_g"][:, 1:2], sb["lh1"][:, :], sb["h"][0:64, :], start=False, stop=True),
           reads=("lh1", "h"), writes=("ps_g",))
        op("act", lambda e: e.activation(sb["tS"][:, 0:2], ps["ps_g"][:, 0:2], AF.Tanh),
           reads=("ps_g",), writes=("tS",))
        # a2 = (t_i + 1) * t_g   on partitions 64:128
        op("dve", lambda e: e.scalar_tensor_tensor(sb["a2b1"][64:128, 0:1], sb["tS"][64:128, 0:1], 1.0,
                                                   sb["tS"][64:128, 1:2], ALU.add, ALU.mult),
           reads=("tS",), writes=("a2b1",))
        # b1 = (t_f + 1) * s     on partitions 0:64   (s = 2c in PSUM)
        op("dve", lambda e: e.scalar_tensor_tensor(sb["a2b1"][0:64, 0:1], sb["tS"][0:64, 0:1], 1.0,
                                                   ps["ps_s"][:, :], ALU.add, ALU.mult),
           reads=("tS", "ps_s"), writes=("a2b1",))
        # s_new = 0.5*b1 + a2 via constant selector matmul
        op("pe", lambda e: e.matmul(ps["ps_s"][:, :], sb["comb"][:, :], sb["a2b1"][:, :], start=True, stop=True),
           reads=("comb", "a2b1"), writes=("ps_s",))
        op("act", lambda e: e.activation(sb["th"][:, :], ps["ps_s"][:, :], AF.Tanh, scale=0.5),
           reads=("ps_s",), writes=("th",))
        # h_dbl = (t_o + 1) * tanh(c2)
        op("dve", lambda e: e.scalar_tensor_tensor(sb["h"][0:64, 0:1], sb["tS"][0:64, 1:2], 1.0,
                                                   sb["th"][:, :], ALU.add, ALU.mult),
           reads=("tS", "th"), writes=("h",))

    def sample_tail(L, step, pos):
        # softmax stats + arc/idx bookkeeping (all off the critical path)
        op("dve", lambda e: e.max_index(sb["ix8"][0:1, 0:8], sb["mx8"][0:1, 0:8], sb["glg8"][0:1, 0:8]),
           reads=("mx8", "glg8"), writes=("ix8",))
        op("dve", lambda e: e.tensor_copy(sb["arcrow"][0:1, pos:pos + 1], sb["ix8"][0:1, 0:1]),
           reads=("ix8",), writes=("arcrow",))
        op("act", lambda e: e.activation(sb["e8"][0:1, 0:L], sb["lg8"][0:1, 0:L], AF.Exp,
                                         accum_out=sb["S"][0:1, 0:1]),
           reads=("lg8",), writes=("e8", "S"))
        op("dve", lambda e: e.reciprocal(sb["rS"][:, :], sb["S"][:, :]),
           reads=("S",), writes=("rS",))
        op("dve", lambda e: e.scalar_tensor_tensor(sb["dscr"][0:1, 0:L], sb["e8"][0:1, 0:L], sb["rS"][0:1, 0:1],
                                                   sb["lg8"][0:1, 0:L], ALU.mult, ALU.mult,
                                                   accum_out=sb["dotn"][0:1, 0:1]),
           reads=("e8", "rS", "lg8"), writes=("dscr", "dotn"))
        op("dve", lambda e: e.scalar_tensor_tensor(sb["ENT"][:, :], sb["dotn"][:, :], -1.0,
                                                   sb["ENT"][:, :], ALU.mult, ALU.add),
           reads=("dotn", "ENT"), writes=("ENT",))
        op("dve", lambda e: e.scalar_tensor_tensor(sb["PS"][:, :], sb["PS"][:, :], sb["S"][0:1, 0:1],
                                                   sb["consts"][0:1, 0:1], ALU.mult, ALU.mult),
           reads=("PS", "S", "consts"), writes=("PS",))
        # LG -= lg[idx] via one-hot dot
        op("dve", lambda e: e.scalar_tensor_tensor(sb["dscr"][0:1, 0:8], sb["oh8"][0:1, 0:8], -1.0,
                                                   sb["lg8"][0:1, 0:8], ALU.mult, ALU.mult,
                                                   accum_out=sb["sel"][0:1, 0:1]),
           reads=("oh8", "lg8"), writes=("dscr", "sel"))
        op("dve", lambda e: e.scalar_tensor_tensor(sb["LG"][:, :], sb["sel"][:, :], 1.0,
                                                   sb["LG"][:, :], ALU.mult, ALU.add),
           reads=("sel", "LG"), writes=("LG",))

    def argmax_onehot(step):
        # glg = lg + gumbel[step]; onehot row = (glg == max); column via K=1 matmul
        op("dve", lambda e: e.tensor_add(sb["glg8"][0:1, 0:8], sb["lg8"][0:1, 0:8],
                                         sb["G"][0:1, 8 * step:8 * step + 8]),
           reads=("lg8", "G"), writes=("glg8",))
        op("dve", lambda e: e.max(sb["mx8"][0:1, 0:8], sb["glg8"][0:1, 0:8]),
           reads=("glg8",), writes=("mx8",))
        op("dve", lambda e: e.tensor_scalar(sb["oh8"][0:1, 0:8], sb["glg8"][0:1, 0:8],
                                            sb["mx8"][0:1, 0:1], scalar2=None, op0=ALU.is_equal),
           reads=("glg8", "mx8"), writes=("oh8",))
        op("pe", lambda e: e.matmul(ps["ps_oh"][:, :], sb["oh8"][0:1, 0:8], sb["consts"][0:1, 1:2],
                                    start=True, stop=True),
           reads=("oh8", "consts"), writes=("ps_oh",))
        op("act", lambda e: e.activation(sb["sboh"][:, :], ps["ps_oh"][:, :], AF.Copy),
           reads=("ps_oh",), writes=("sboh",))

    def index_step(L, step, pos):
        cell()
        op("pe", lambda e: e.matmul(ps["ps_hw2"][:, :], sb["w2T"][:, :], sb["h"][0:64, :], start=True, stop=True),
           reads=("w2T", "h"), writes=("ps_hw2",))
        op("act", lambda e: e.activation(sb["hw2"][:, :], ps["ps_hw2"][:, :], AF.Copy),
           reads=("ps_hw2",), writes=("hw2",))
        op("act", lambda e: e.activation(sb["q"][:, 0:L], sb["aw1"][:, 0:L], AF.Tanh, bias=sb["hw2"][:, 0:1]),
           reads=("aw1", "hw2"), writes=("q",))
        op("pe", lambda e: e.matmul(ps["ps_row"][0:1, 0:L], sb["v5"][:, :], sb["q"][:, 0:L], start=True, stop=True),
           reads=("v5", "q"), writes=("ps_row",))
        op("act", lambda e: e.activation(sb["t1row"][0:1, 0:L], ps["ps_row"][0:1, 0:L], AF.Tanh, scale=5.0),
           reads=("ps_row",), writes=("t1row",))
        op("dve", lambda e: e.scalar_tensor_tensor(sb["lg8"][0:1, 0:L], sb["t1row"][0:1, 0:L], 1.1,
                                                   ps["ps_row"][0:1, 0:L], ALU.mult, ALU.add),
           reads=("t1row", "ps_row"), writes=("lg8",))
        argmax_onehot(step)
        op("pe", lambda e: e.matmul(ps["ps_x"][:, :], sb["anchrows"][:, :], sb["sboh"][:, :],
                                    start=True, stop=True),
           reads=("anchrows", "sboh"), writes=("ps_x",))
        op("act", lambda e: e.activation(sb["x"][0:64, 0:1], ps["ps_x"][:, :], AF.Copy),
           reads=("ps_x",), writes=("x",))
        sample_tail(L, step, pos)

    def op_step(samp, step, pos):
        cell()
        op("pe", lambda e: e.matmul(ps["ps_row"][0:1, 0:5], sb["h"][:, :], sb["wsoft5"][:, :], start=True, stop=True),
           reads=("h", "wsoft5"), writes=("ps_row",))
        op("act", lambda e: e.activation(sb["t1row"][0:1, 0:5], ps["ps_row"][0:1, 0:5], AF.Tanh),
           reads=("ps_row",), writes=("t1row",))
        boff = 8 * samp
        op("dve", lambda e: e.scalar_tensor_tensor(sb["lg8"][0:1, 0:5], sb["t1row"][0:1, 0:5], float(1.1 / 2.5),
                                                   sb["brow"][0:1, boff:boff + 5], ALU.mult, ALU.add),
           reads=("t1row", "brow"), writes=("lg8",))
        argmax_onehot(step)
        op("pe", lambda e: e.matmul(ps["ps_x"][:, :], sb["embT1"][:, :], sb["sboh"][0:5, :],
                                    start=True, stop=True),
           reads=("embT1", "sboh"), writes=("ps_x",))
        op("act", lambda e: e.activation(sb["x"][0:64, 0:1], ps["ps_x"][:, :], AF.Copy),
           reads=("ps_x",), writes=("x",))
        sample_tail(5, step, pos)

    def store_anchor(slot):
        # anchor row slot = 0.5*h_dbl^T, accumulated as a unit-row outer product
        # into a persistent [8,64] PSUM bank (slot 2 restarts the group), then
        # snapshot the whole block to SBUF for use as matmul weights.
        op("pe", lambda e: e.transpose(ps["ps_t"][:, :], sb["h"][0:64, 0:1], sb["iden"][:, :]),
           reads=("h", "iden"), writes=("ps_t",))
        op("act", lambda e: e.activation(sb["hrow"][:, :], ps["ps_t"][:, :], AF.Copy, scale=0.5),
           reads=("ps_t",), writes=("hrow",))
        op("pe", lambda e: e.matmul(ps["ps_rows"][:, :], sb["urow"][0:1, 8 * slot:8 * slot + 8],
                                    sb["hrow"][:, :], start=True, stop=True),
           reads=("urow", "hrow"), writes=("ps_rows",))
        if slot == 2:
            op("dve", lambda e: e.tensor_copy(sb["anchrows"][:, :], ps["ps_rows"][:, :]),
               reads=("ps_rows",), writes=("anchrows",))
        else:
            op("dve", lambda e: e.tensor_add(sb["anchrows"][:, :], sb["anchrows"][:, :], ps["ps_rows"][:, :]),
               reads=("anchrows", "ps_rows"), writes=("anchrows",))
        op("pe", lambda e: e.matmul(ps["ps_hw2"][:, :], sb["w1T"][:, :], sb["h"][0:64, :], start=True, stop=True),
           reads=("w1T", "h"), writes=("ps_hw2",))
        op("act", lambda e: e.activation(sb["aw1"][:, slot:slot + 1], ps["ps_hw2"][:, :], AF.Copy),
           reads=("ps_hw2",), writes=("aw1",))

    # --- full schedule ---
    step = 0
    for samp in range(2):
        base = 20 * samp
        op("dve", lambda e: e.tensor_copy(sb["x"][0:64, 0:1], sb["emb"][:, 0:1]),
           reads=("emb",), writes=("x",))
        for k in range(2):
            cell()
            op("pe", lambda e: e.matmul(ps["ps_hw2"][:, :], sb["w1T"][:, :], sb["h"][0:64, :], start=True, stop=True),
               reads=("w1T", "h"), writes=("ps_hw2",))
            def aw(k=k):
                return lambda e: e.activation(sb["aw1"][:, k:k + 1], ps["ps_hw2"][:, :], AF.Copy)
            op("act", aw(), reads=("ps_hw2",), writes=("aw1",))
        for lid in range(2, 7):
            for i in range(2):
                index_step(lid, step, base + 4 * (lid - 2) + 2 * i)
                step += 1
            for i in range(2):
                op_step(samp, step, base + 4 * (lid - 2) + (1 if i == 0 else 3))
                step += 1
            cell()
            store_anchor(lid)
            op("dve", lambda e: e.tensor_copy(sb["x"][0:64, 0:1], sb["emb"][:, 0:1]),
               reads=("emb",), writes=("x",))
    assert step == 40

    # --- final: lp / ent ---
    C = float(-40.0 * np.log(KSCALE))
    op("act", lambda e: e.activation(sb["lnPS"][:, :], sb["PS"][:, :], AF.Ln),
       reads=("PS",), writes=("lnPS",))
    op("dve", lambda e: e.scalar_tensor_tensor(sb["scal"][0:1, 0:1], sb["lnPS"][:, :], C,
                                               sb["LG"][:, :], ALU.add, ALU.add),
       reads=("lnPS", "LG"), writes=("scal",))
    op("dve", lambda e: e.scalar_tensor_tensor(sb["scal"][0:1, 1:2], sb["lnPS"][:, :], C,
                                               sb["ENT"][:, :], ALU.add, ALU.add),
       reads=("lnPS", "ENT"), writes=("scal",))
    import concourse.mybir as _mb
    op("sp", lambda e: e.dma_start(out=arc_out[None, :], in_=sb["arcrow"][0:1, 0:40].bitcast(_mb.dt.int32)),
       reads=("arcrow",), writes=())
    op("sp", lambda e: e.dma_start(out=scal_out[None, :], in_=sb["scal"][0:1, 0:2]),
       reads=("scal",), writes=())

    # ---- phase 1.5: dependency analysis -> waits ----
    count = {e: 0 for e in ENGS}
    count["dma"] = 0
    last_write = {}           # buf -> (eng, cnt)
    readers = {}              # buf -> {eng: cnt}
    INC = {"sp": 16}          # DMA sem increments (sp ops are DMAs here)
    for o in ops:
        eng = o["eng"]
        semeng = "dma" if eng == "sp" else eng
        inc = INC.get(eng, 1)
        waits = {}            # producer sem -> min count
        def add_wait(peng, pcnt):
            same = peng == semeng or (peng == eng and eng != "sp")
            if same and eng not in strict_engs and not o["self_wait"]:
                return
            waits[peng] = max(waits.get(peng, 0), pcnt)
        for buf in o["reads"]:
            if buf in last_write:
                add_wait(*last_write[buf])
        for buf in o["writes"]:
            if buf in last_write:
                add_wait(*last_write[buf])
            for peng, pcnt in readers.get(buf, {}).items():
                add_wait(peng, pcnt)
        if o["self_wait"]:
            # force: wait for all prior same-engine ops to fully retire
            waits[semeng] = max(waits.get(semeng, 0), count[semeng])
        if not o["no_inc"]:
            count[semeng] += inc
        o["idx"] = count[semeng]
        o["waits"] = sorted(waits.items())
        for buf in o["reads"]:
            readers.setdefault(buf, {})[semeng] = count[semeng]
        for buf in o["writes"]:
            last_write[buf] = (semeng, count[semeng])
            readers[buf] = {}

    # HWDGE coalesces back-to-back sem updates in one FIFO: per-DMA wait
    # values are not guaranteed update points.  All input loads are issued
    # back-to-back, so round any wait on the input batch up to its total.
    n_input_loads = len(ins)
    dma_in_total = 16 * n_input_loads
    for o in ops:
        o["waits"] = [(p, dma_in_total if (p == "dma" and c <= dma_in_total) else c)
                      for (p, c) in o["waits"]]

    # ---- phase 2: emit per-engine streams ----
    EMAP = {"pe": "tensor", "act": "scalar", "dve": "vector", "sp": "sync"}
    waited = {e: {p: 0 for p in sems} for e in ENGS}

    def emit_stream(eng_name):
        def closure(engine):
            for o in ops:
                if o["eng"] != eng_name:
                    continue
                for peng, pcnt in o["waits"]:
                    if waited[eng_name][peng] < pcnt:
                        engine.wait_ge(sems[peng], pcnt)
                        waited[eng_name][peng] = pcnt
                inst = o["fn"](engine)
                if inst is not None and not o["no_inc"]:
                    semeng = "dma" if eng_name == "sp" else eng_name
                    inst.then_inc(sems[semeng], INC.get(eng_name, 1))
            if eng_name == "sp":
                # ensure output DMAs have landed before the end-of-kernel barrier
                engine.wait_ge(sems["dma"], count["dma"])
        return closure

    with nc.Block() as block:
        block.tensor(emit_stream("pe"))
        block.scalar(emit_stream("act"))
        block.vector(emit_stream("dve"))
        block.sync(emit_stream("sp"))

    stack.close()
    return nc


_CACHE = {}

def _postprocess(res):
    arc = np.asarray(res["arc"]).reshape(40).astype(np.int32)
    scal = np.asarray(res["scal"]).reshape(2).astype(np.float32)
    return (arc[:20].copy(), arc[20:].copy(),
            np.float32(scal[0]), np.float32(scal[1]))


def kernel(**inputs):
    from concourse.bass_utils import run_bass_kernel_spmd
    pin = prep_inputs(inputs)
    if "nc" not in _CACHE:
        # {"act","dve"} same-engine waits are required for correctness on HW
        # (engine pipelines overlap back-to-back ops); PE/SP run in order.
        _CACHE["nc"] = build_program(strict={"act", "dve"})
    nc = _CACHE["nc"]
    core_ids = list(range(8))
    in_maps = [dict(pin) for _ in core_ids]
    out = run_bass_kernel_spmd(nc, in_maps, core_ids)
    return _postprocess(out.results[0])
